# revision 4
# baseline (speedup 1.0000x reference)
# Mistral sliding-window attention (B=1, S=2048, H=4096, 32 q heads / 8 kv
# heads, window 4096 -> plain causal at this S) on 8 Trainium2 NeuronCores.
#
# Sharding: tensor-parallel over heads. Core c owns q heads 4c..4c+3 and kv
# head c. hidden_states is replicated (transposed on host to [H, S] so the
# contraction dim is the partition dim). Each core computes its attention
# output slice attn.T [512, S], one AllGather assembles the full [4096, S],
# and each core then computes a 512-column slice of o_proj; the host
# concatenates the 8 column slices into the full output.
#
# All big matmuls run as float32r (fp32 storage, full-rate PE) with the
# moving dim = 512. Scores are computed transposed (S.T[kv, q]) so that the
# P@V contraction needs no transposes of the probability tiles; softmax
# denominators come from an all-ones stationary matmul accumulated alongside
# P@V, and the sliding-window/causal mask is a host-precomputed staircase
# slice multiplied in after exp.

from contextlib import ExitStack

import numpy as np

import concourse.bacc as bacc
import concourse.bass as bass
import concourse.mybir as mybir
import concourse.tile as tile
from concourse.bass_utils import run_bass_kernel_spmd
from concourse.masks import make_identity

HIDDEN = 4096
NH = 32
NKV = 8
HD = 128
THETA = 10000.0
S = 2048
NCORES = 8

QH = NH // NCORES          # 4 q heads per core
DQ = QH * HD               # 512 (per-core q/attn width)
DOUT = DQ + 2 * HD         # 768 = q heads + k + v projection width
MT = DOUT // 128           # 6 projection m-tiles (0..3 q, 4 k, 5 v)
KT = HIDDEN // 128         # 32 contraction tiles
TCH = 512                  # token chunk (matmul moving dim)
NTCH = S // TCH            # 4
KVT = S // 128             # 16 kv tiles
SCALE = 1.0 / float(np.sqrt(HD))

F32 = mybir.dt.float32
F32R = mybir.dt.float32r
EXP = mybir.ActivationFunctionType.Exp


def _rope(nc, pool, src, dst, cs, sn, n):
    """dst = src*cos + rotate_half(src)*sin, in [d, tok] layout.

    src/dst are [128, n]; cs/sn are [64, n] (the two 64-row halves share
    frequencies). rotate_half: rows 0:64 get -src[64:128], rows 64:128 get
    src[0:64].
    """
    top, bot = src[0:64, :], src[64:128, :]
    ta = pool.tile([64, TCH], F32, name="rope_a")[:, :n]
    tb = pool.tile([64, TCH], F32, name="rope_b")[:, :n]
    nc.vector.tensor_mul(ta, top, cs)
    nc.vector.tensor_mul(tb, bot, sn)
    nc.vector.tensor_sub(dst[0:64, :], ta, tb)
    nc.vector.tensor_mul(ta, bot, cs)
    nc.vector.tensor_mul(tb, top, sn)
    nc.vector.tensor_add(dst[64:128, :], ta, tb)


def build_kernel_body(ctx: ExitStack, tc: tile.TileContext, outs, ins):
    nc = tc.nc
    xT, wqkv, ow, cos_t, sin_t, stair = (
        ins["xT"], ins["wqkv"], ins["ow"], ins["cos_t"], ins["sin_t"], ins["stair"],
    )
    out = outs["out"]

    attn_loc = nc.dram_tensor("attn_loc", [DQ, S], F32).ap()
    attn_gat = nc.dram_tensor("attn_gat", [HIDDEN, S], F32, addr_space="Shared").ap()

    singles = ctx.enter_context(tc.tile_pool(name="singles", bufs=1))
    cos_sb = singles.tile([64, S], F32)
    sin_sb = singles.tile([64, S], F32)
    stair_sb = singles.tile([128, 896], F32)
    nc.sync.dma_start(out=cos_sb, in_=cos_t)
    nc.sync.dma_start(out=sin_sb, in_=sin_t)
    nc.sync.dma_start(out=stair_sb, in_=stair)
    ones_sb = singles.tile([128, 128], F32R)
    ones_f = singles.tile([128, 128], F32)
    nc.vector.memset(ones_f, 1.0)
    nc.vector.tensor_copy(ones_sb, ones_f)
    ident_sb = singles.tile([128, 128], F32)
    make_identity(nc, ident_sb)

    # persistent projection outputs, [d, tok] layout
    qT = singles.tile([128, QH, S], F32R)    # q head h -> qT[:, h, :]
    kT = singles.tile([128, S], F32R)
    vT = singles.tile([128, S], F32)
    V = singles.tile([128, KVT, HD], F32R)   # V[:, j, :] = [tok 128, d 128]

    # ---- phase 1: QKV projection + RoPE --------------------------------
    with (
        tc.tile_pool(name="wq", bufs=1) as wp,
        tc.tile_pool(name="xt", bufs=3) as xp,
        tc.tile_pool(name="rope", bufs=4) as rp,
        tc.tile_pool(name="p1ps", bufs=1, space="PSUM") as pp1,
    ):
        w_sb = wp.tile([128, KT, DOUT], F32R)
        nc.sync.dma_start(out=w_sb, in_=wqkv.rearrange("(k p) d -> p k d", p=128))
        for t in range(NTCH):
            ps = [pp1.tile([128, TCH], F32, name=f"p1_{m}", tag=f"p1_{m}")
                  for m in range(MT)]
            for k in range(KT):
                xt = xp.tile([128, TCH], F32R, name="xt")
                nc.sync.dma_start(
                    out=xt, in_=xT[k * 128:(k + 1) * 128, t * TCH:(t + 1) * TCH])
                for m in range(MT):
                    nc.tensor.matmul(
                        ps[m],
                        lhsT=w_sb[:, k, m * 128:(m + 1) * 128],
                        rhs=xt,
                        start=(k == 0), stop=(k == KT - 1),
                    )
            cs = cos_sb[:, t * TCH:(t + 1) * TCH]
            sn = sin_sb[:, t * TCH:(t + 1) * TCH]
            for h in range(QH):
                _rope(nc, rp, ps[h], qT[:, h, t * TCH:(t + 1) * TCH], cs, sn, TCH)
            _rope(nc, rp, ps[QH], kT[:, t * TCH:(t + 1) * TCH], cs, sn, TCH)
            nc.scalar.copy(out=vT[:, t * TCH:(t + 1) * TCH], in_=ps[QH + 1])

        # V = vT.T per kv tile (PE transpose; [d, tok] -> [tok, d])
        for j in range(KVT):
            pv = pp1.tile([128, 128], F32, name="pvt", tag="pvt")
            nc.tensor.transpose(pv, vT[:, j * 128:(j + 1) * 128], ident_sb)
            nc.scalar.copy(out=V[:, j, :], in_=pv)

    # ---- phase 2: attention (scores transposed: [kv, q]) ---------------
    with (
        tc.tile_pool(name="pt", bufs=4) as ptp,
        tc.tile_pool(name="ao", bufs=4) as aop,
        tc.tile_pool(name="p2sc", bufs=2, space="PSUM") as pp2,
        tc.tile_pool(name="p2acc", bufs=2, space="PSUM") as pa2,
    ):
        for h in range(QH):
            for c in range(NTCH):
                jmax = 4 * c + 3
                po = pa2.tile([128, TCH], F32, name="po", tag="po")
                psum_s = pa2.tile([128, TCH], F32, name="ps", tag="ps")
                qslice = qT[:, h, c * TCH:(c + 1) * TCH]
                for j in range(jmax + 1):
                    sc = pp2.tile([128, TCH], F32, name="sc", tag="sc")
                    nc.tensor.matmul(
                        sc, lhsT=kT[:, j * 128:(j + 1) * 128], rhs=qslice,
                        start=True, stop=True)
                    pt = ptp.tile([128, TCH], F32R, name="pt")
                    nc.scalar.activation(pt, sc, EXP, scale=SCALE)
                    rdiag = j - 4 * c
                    if rdiag >= 0:  # tile touches the causal diagonal
                        off = 384 - rdiag * 128
                        nc.vector.tensor_mul(pt, pt, stair_sb[:, off:off + TCH])
                    nc.tensor.matmul(po, lhsT=V[:, j, :], rhs=pt,
                                     start=(j == 0), stop=(j == jmax))
                    nc.tensor.matmul(psum_s, lhsT=ones_sb, rhs=pt,
                                     start=(j == 0), stop=(j == jmax))
                rec = aop.tile([128, TCH], F32, name="rec")
                nc.vector.reciprocal(rec, psum_s)
                ao = aop.tile([128, TCH], F32, name="ao")
                nc.vector.tensor_mul(ao, po, rec)
                nc.sync.dma_start(
                    out=attn_loc[h * 128:(h + 1) * 128, c * TCH:(c + 1) * TCH],
                    in_=ao)

    # ---- phase 2.5: AllGather attn.T across the 8 cores ----------------
    nc.gpsimd.collective_compute(
        "AllGather",
        mybir.AluOpType.bypass,
        ins=[attn_loc[:, :]],
        outs=[attn_gat[:, :]],
        replica_groups=[list(range(NCORES))],
    )

    # ---- phase 3: o_proj column slice ----------------------------------
    with (
        tc.tile_pool(name="ow", bufs=1) as owp,
        tc.tile_pool(name="at", bufs=3) as atp,
        tc.tile_pool(name="oc", bufs=3) as ocp,
        tc.tile_pool(name="p3ps", bufs=1, space="PSUM") as pp3,
    ):
        ow_sb = owp.tile([128, KT, DQ], F32R)
        nc.sync.dma_start(out=ow_sb, in_=ow.rearrange("(k p) d -> p k d", p=128))
        for g in range(S // TCH):
            pc = [pp3.tile([128, TCH], F32, name=f"p3_{mi}", tag=f"p3_{mi}")
                  for mi in range(4)]
            for k in range(KT):
                at = atp.tile([128, TCH], F32R, name="at")
                nc.sync.dma_start(
                    out=at,
                    in_=attn_gat[k * 128:(k + 1) * 128, g * TCH:(g + 1) * TCH].bitcast(F32R))
                for mi in range(4):
                    nc.tensor.matmul(
                        pc[mi],
                        lhsT=at[:, mi * 128:(mi + 1) * 128],
                        rhs=ow_sb[:, k, :],
                        start=(k == 0), stop=(k == KT - 1),
                    )
            for mi in range(4):
                oc = ocp.tile([128, TCH], F32, name="oc")
                nc.scalar.copy(out=oc, in_=pc[mi])
                nc.sync.dma_start(
                    out=out[(g * 4 + mi) * 128:(g * 4 + mi + 1) * 128, :], in_=oc)


_NC_CACHE = None


def build_program():
    global _NC_CACHE
    if _NC_CACHE is not None:
        return _NC_CACHE
    nc = bacc.Bacc("TRN2", target_bir_lowering=False, debug=False,
                   num_devices=NCORES)
    ins = {
        "xT": nc.dram_tensor("xT", [HIDDEN, S], F32R, kind="ExternalInput").ap(),
        "wqkv": nc.dram_tensor("wqkv", [HIDDEN, DOUT], F32R,
                               kind="ExternalInput").ap(),
        "ow": nc.dram_tensor("ow", [HIDDEN, DQ], F32R, kind="ExternalInput").ap(),
        "cos_t": nc.dram_tensor("cos_t", [64, S], F32, kind="ExternalInput").ap(),
        "sin_t": nc.dram_tensor("sin_t", [64, S], F32, kind="ExternalInput").ap(),
        "stair": nc.dram_tensor("stair", [128, 896], F32,
                                kind="ExternalInput").ap(),
    }
    outs = {"out": nc.dram_tensor("out", [S, DQ], F32, kind="ExternalOutput").ap()}
    with tile.TileContext(nc) as tc:
        with ExitStack() as ctx:
            build_kernel_body(ctx, tc, outs, ins)
    nc.compile()
    _NC_CACHE = nc
    return nc


def make_in_maps(hidden_states, position_ids, q_w, k_w, v_w, o_w):
    x = np.asarray(hidden_states, dtype=np.float32).reshape(S, HIDDEN)
    xT = np.ascontiguousarray(x.T)
    pos = np.asarray(position_ids).reshape(S).astype(np.float64)
    inv = 1.0 / (THETA ** (np.arange(0, HD, 2, dtype=np.float64) / HD))
    fr = inv[:, None] * pos[None, :]                       # [64, S]
    cos_t = np.cos(fr).astype(np.float32)
    sin_t = np.sin(fr).astype(np.float32)
    u = np.arange(896, dtype=np.int64)[None, :]
    kvi = np.arange(128, dtype=np.int64)[:, None]
    stair = ((u - kvi) >= 384).astype(np.float32)          # [128, 896]

    q_w = np.asarray(q_w, dtype=np.float32)
    k_w = np.asarray(k_w, dtype=np.float32)
    v_w = np.asarray(v_w, dtype=np.float32)
    o_w = np.asarray(o_w, dtype=np.float32)

    in_maps = []
    for c in range(NCORES):
        wqkv = np.ascontiguousarray(np.concatenate(
            [q_w[:, c * DQ:(c + 1) * DQ],
             k_w[:, c * HD:(c + 1) * HD],
             v_w[:, c * HD:(c + 1) * HD]], axis=1))
        owc = np.ascontiguousarray(o_w[:, c * DQ:(c + 1) * DQ])
        in_maps.append({"xT": xT, "wqkv": wqkv, "ow": owc,
                        "cos_t": cos_t, "sin_t": sin_t, "stair": stair})
    return in_maps


def run(inputs: dict, trace: bool = False):
    """Run on the 8 NeuronCores; returns (full_output, BassKernelResults)."""
    nc = build_program()
    in_maps = make_in_maps(**inputs)
    res = run_bass_kernel_spmd(nc, in_maps, core_ids=list(range(NCORES)),
                               trace=trace)
    full = np.concatenate([res.results[c]["out"] for c in range(NCORES)], axis=1)
    return full.reshape(1, S, HIDDEN), res


def kernel(**inputs) -> np.ndarray:
    out, _ = run(inputs)
    return out


# revision 9
# speedup vs baseline: 1.0582x; 1.0582x over previous
# Mistral sliding-window attention (B=1, S=2048, H=4096, 32 q heads / 8 kv
# heads, window 4096 -> plain causal at this S) on 8 Trainium2 NeuronCores.
#
# Sharding: tensor-parallel over heads. Core c owns q heads 4c..4c+3 and kv
# head c. hidden_states is replicated (transposed on host to [H, S] so the
# contraction dim is the partition dim). Each core computes its attention
# output slice attn.T [512, S], one AllGather assembles the full [4096, S],
# and each core then computes a 512-column slice of o_proj; the host
# concatenates the 8 column slices into the full output.
#
# All big matmuls run as float32r (fp32 storage, full-rate PE) with the
# moving dim = 512. Scores are computed transposed (S.T[kv, q]) so that the
# P@V contraction needs no transposes of the probability tiles; softmax
# denominators come from an all-ones stationary matmul accumulated alongside
# P@V, and the sliding-window/causal mask is a host-precomputed staircase
# slice multiplied in after exp.

from contextlib import ExitStack

import numpy as np

import concourse.bacc as bacc
import concourse.bass as bass
import concourse.mybir as mybir
import concourse.tile as tile
from concourse.bass_utils import run_bass_kernel_spmd
from concourse.masks import make_identity

HIDDEN = 4096
NH = 32
NKV = 8
HD = 128
THETA = 10000.0
S = 2048
NCORES = 8

QH = NH // NCORES          # 4 q heads per core
DQ = QH * HD               # 512 (per-core q/attn width)
DOUT = DQ + 2 * HD         # 768 = q heads + k + v projection width
MT = DOUT // 128           # 6 projection m-tiles (0..3 q, 4 k, 5 v)
KT = HIDDEN // 128         # 32 contraction tiles
TCH = 512                  # token chunk (matmul moving dim)
NTCH = S // TCH            # 4
KVT = S // 128             # 16 kv tiles
SCALE = 1.0 / float(np.sqrt(HD))

F32 = mybir.dt.float32
F32R = mybir.dt.float32r
EXP = mybir.ActivationFunctionType.Exp


def _rope(nc, pool, src, dst, cs, sn, n):
    """dst = src*cos + rotate_half(src)*sin, in [d, tok] layout.

    src/dst are [128, n]; cs/sn are [64, n] (the two 64-row halves share
    frequencies). rotate_half: rows 0:64 get -src[64:128], rows 64:128 get
    src[0:64].
    """
    top, bot = src[0:64, :], src[64:128, :]
    ta = pool.tile([64, TCH], F32, name="rope_a")[:, :n]
    tb = pool.tile([64, TCH], F32, name="rope_b")[:, :n]
    nc.vector.tensor_mul(ta, top, cs)
    nc.vector.tensor_mul(tb, bot, sn)
    nc.vector.tensor_sub(dst[0:64, :], ta, tb)
    nc.vector.tensor_mul(ta, bot, cs)
    nc.vector.tensor_mul(tb, top, sn)
    nc.vector.tensor_add(dst[64:128, :], ta, tb)


def build_kernel_body(ctx: ExitStack, tc: tile.TileContext, outs, ins):
    nc = tc.nc
    xT, wqkv, ow, cos_t, sin_t, stair = (
        ins["xT"], ins["wqkv"], ins["ow"], ins["cos_t"], ins["sin_t"], ins["stair"],
    )
    out = outs["out"]

    # per-head bounce + gather buffers so each head's AllGather can fire as
    # soon as that head's attention is done (overlaps comm with compute)
    attn_loc = [nc.dram_tensor(f"attn_loc{h}", [HD, S], F32).ap()
                for h in range(QH)]
    attn_gat = [nc.dram_tensor(f"attn_gat{h}", [NCORES * HD, S], F32,
                               addr_space="Shared").ap()
                for h in range(QH)]

    singles = ctx.enter_context(tc.tile_pool(name="singles", bufs=1))
    cos_sb = singles.tile([64, S], F32)
    sin_sb = singles.tile([64, S], F32)
    stair_sb = singles.tile([128, 896], F32)
    nc.sync.dma_start(out=cos_sb, in_=cos_t)
    nc.sync.dma_start(out=sin_sb, in_=sin_t)
    nc.sync.dma_start(out=stair_sb, in_=stair)
    ones_sb = singles.tile([128, 128], F32R)
    ones_f = singles.tile([128, 128], F32)
    nc.vector.memset(ones_f, 1.0)
    nc.vector.tensor_copy(ones_sb, ones_f)
    ident_sb = singles.tile([128, 128], F32)
    make_identity(nc, ident_sb)

    # persistent projection outputs, [d, tok] layout
    qT = singles.tile([128, QH, S], F32R)    # q head h -> qT[:, h, :]
    kT = singles.tile([128, S], F32R)
    vT = singles.tile([128, S], F32)
    V = singles.tile([128, KVT, HD], F32R)   # V[:, j, :] = [tok 128, d 128]

    # ---- phase 1: QKV projection + RoPE --------------------------------
    with (
        tc.tile_pool(name="wq", bufs=1) as wp,
        tc.tile_pool(name="xt", bufs=3) as xp,
        tc.tile_pool(name="rope", bufs=4) as rp,
        tc.tile_pool(name="p1ps", bufs=1, space="PSUM") as pp1,
    ):
        # one DMA per k-tile so the first matmul starts after ~1/32 of the
        # weight load instead of after the full 12 MB
        wq3 = wqkv.rearrange("(k p) d -> p k d", p=128)
        w_sb = []
        for k in range(KT):
            wk = wp.tile([128, DOUT], F32R, name=f"w{k}", tag=f"w{k}")
            nc.sync.dma_start(out=wk, in_=wq3[:, k, :])
            w_sb.append(wk)
        for t in range(NTCH):
            ps = [pp1.tile([128, TCH], F32, name=f"p1_{m}", tag=f"p1_{m}")
                  for m in range(MT)]
            for k in range(KT):
                xt = xp.tile([128, TCH], F32R, name="xt")
                nc.sync.dma_start(
                    out=xt, in_=xT[k * 128:(k + 1) * 128, t * TCH:(t + 1) * TCH])
                for m in range(MT):
                    nc.tensor.matmul(
                        ps[m],
                        lhsT=w_sb[k][:, m * 128:(m + 1) * 128],
                        rhs=xt,
                        start=(k == 0), stop=(k == KT - 1),
                    )
            cs = cos_sb[:, t * TCH:(t + 1) * TCH]
            sn = sin_sb[:, t * TCH:(t + 1) * TCH]
            for h in range(QH):
                _rope(nc, rp, ps[h], qT[:, h, t * TCH:(t + 1) * TCH], cs, sn, TCH)
            _rope(nc, rp, ps[QH], kT[:, t * TCH:(t + 1) * TCH], cs, sn, TCH)
            nc.scalar.copy(out=vT[:, t * TCH:(t + 1) * TCH], in_=ps[QH + 1])
            # V = vT.T for this chunk's kv tiles (PE transpose [d,tok]->[tok,d])
            for j in range(4 * t, 4 * t + 4):
                pv = pp1.tile([128, 128], F32, name="pvt", tag="pvt")
                nc.tensor.transpose(pv, vT[:, j * 128:(j + 1) * 128], ident_sb)
                nc.scalar.copy(out=V[:, j, :], in_=pv)

    # ---- phases 2+3 share pools so o_proj matmuls can interleave with the
    # attention tail while the per-head AllGathers are in flight ----------
    with (
        tc.tile_pool(name="pt", bufs=4) as ptp,
        tc.tile_pool(name="ao", bufs=4) as aop,
        tc.tile_pool(name="ow", bufs=1) as owp,
        tc.tile_pool(name="at", bufs=3) as atp,
        tc.tile_pool(name="oc", bufs=3) as ocp,
        tc.tile_pool(name="p2sc", bufs=2, space="PSUM") as pp2,
        tc.tile_pool(name="p2acc", bufs=1, space="PSUM") as pa2,
        tc.tile_pool(name="p3ps", bufs=1, space="PSUM") as pp3,
    ):
        # o_proj weights, one DMA per k-tile; k-tile kk = r*QH + h holds
        # o_w rows for (rank r, head h)
        ow3 = ow.rearrange("(k p) d -> p k d", p=128)
        ow_sb = []
        for k in range(KT):
            owk = owp.tile([128, DQ], F32R, name=f"ow{k}", tag=f"ow{k}")
            nc.sync.dma_start(out=owk, in_=ow3[:, k, :])
            ow_sb.append(owk)

        # ---- attention (scores transposed: [kv, q]), AG per head -------
        for h in range(QH):
            for c in range(NTCH):
                jmax = 4 * c + 3
                po = pa2.tile([128, TCH], F32, name="po", tag="po")
                psum_s = pa2.tile([128, TCH], F32, name="ps", tag="ps")
                qslice = qT[:, h, c * TCH:(c + 1) * TCH]
                for j in range(jmax + 1):
                    sc = pp2.tile([128, TCH], F32, name="sc", tag="sc")
                    nc.tensor.matmul(
                        sc, lhsT=kT[:, j * 128:(j + 1) * 128], rhs=qslice,
                        start=True, stop=True)
                    pt = ptp.tile([128, TCH], F32R, name="pt")
                    nc.scalar.activation(pt, sc, EXP, scale=SCALE)
                    rdiag = j - 4 * c
                    if rdiag >= 0:  # tile touches the causal diagonal
                        off = 384 - rdiag * 128
                        nc.vector.tensor_mul(pt, pt, stair_sb[:, off:off + TCH])
                    nc.tensor.matmul(po, lhsT=V[:, j, :], rhs=pt,
                                     start=(j == 0), stop=(j == jmax))
                    nc.tensor.matmul(psum_s, lhsT=ones_sb, rhs=pt,
                                     start=(j == 0), stop=(j == jmax))
                rec = aop.tile([128, TCH], F32, name="rec")
                nc.vector.reciprocal(rec, psum_s)
                ao = aop.tile([128, TCH], F32, name="ao")
                nc.vector.tensor_mul(ao, po, rec)
                nc.sync.dma_start(
                    out=attn_loc[h][:, c * TCH:(c + 1) * TCH], in_=ao)
            nc.gpsimd.collective_compute(
                "AllGather",
                mybir.AluOpType.bypass,
                ins=[attn_loc[h][:, :]],
                outs=[attn_gat[h][:, :]],
                replica_groups=[list(range(NCORES))],
            )

        # ---- o_proj column slice; h-major k-order so each gathered head
        # can be consumed as soon as its AllGather lands ------------------
        for g in range(S // TCH):
            pc = [pp3.tile([128, TCH], F32, name=f"p3_{mi}", tag=f"p3_{mi}")
                  for mi in range(4)]
            for h in range(QH):
                for r in range(NCORES):
                    at = atp.tile([128, TCH], F32R, name="at")
                    nc.sync.dma_start(
                        out=at,
                        in_=attn_gat[h][r * 128:(r + 1) * 128,
                                        g * TCH:(g + 1) * TCH].bitcast(F32R))
                    first = (h == 0 and r == 0)
                    last = (h == QH - 1 and r == NCORES - 1)
                    for mi in range(4):
                        nc.tensor.matmul(
                            pc[mi],
                            lhsT=at[:, mi * 128:(mi + 1) * 128],
                            rhs=ow_sb[r * QH + h],
                            start=first, stop=last,
                        )
            for mi in range(4):
                oc = ocp.tile([128, TCH], F32, name="oc")
                nc.scalar.copy(out=oc, in_=pc[mi])
                nc.sync.dma_start(
                    out=out[(g * 4 + mi) * 128:(g * 4 + mi + 1) * 128, :], in_=oc)


_NC_CACHE = None


def build_program():
    global _NC_CACHE
    if _NC_CACHE is not None:
        return _NC_CACHE
    nc = bacc.Bacc("TRN2", target_bir_lowering=False, debug=False,
                   num_devices=NCORES)
    ins = {
        "xT": nc.dram_tensor("xT", [HIDDEN, S], F32R, kind="ExternalInput").ap(),
        "wqkv": nc.dram_tensor("wqkv", [HIDDEN, DOUT], F32R,
                               kind="ExternalInput").ap(),
        "ow": nc.dram_tensor("ow", [HIDDEN, DQ], F32R, kind="ExternalInput").ap(),
        "cos_t": nc.dram_tensor("cos_t", [64, S], F32, kind="ExternalInput").ap(),
        "sin_t": nc.dram_tensor("sin_t", [64, S], F32, kind="ExternalInput").ap(),
        "stair": nc.dram_tensor("stair", [128, 896], F32,
                                kind="ExternalInput").ap(),
    }
    outs = {"out": nc.dram_tensor("out", [S, DQ], F32, kind="ExternalOutput").ap()}
    with tile.TileContext(nc) as tc:
        with ExitStack() as ctx:
            build_kernel_body(ctx, tc, outs, ins)
    nc.compile()
    _NC_CACHE = nc
    return nc


def make_in_maps(hidden_states, position_ids, q_w, k_w, v_w, o_w):
    x = np.asarray(hidden_states, dtype=np.float32).reshape(S, HIDDEN)
    xT = np.ascontiguousarray(x.T)
    pos = np.asarray(position_ids).reshape(S).astype(np.float64)
    inv = 1.0 / (THETA ** (np.arange(0, HD, 2, dtype=np.float64) / HD))
    fr = inv[:, None] * pos[None, :]                       # [64, S]
    cos_t = np.cos(fr).astype(np.float32)
    sin_t = np.sin(fr).astype(np.float32)
    u = np.arange(896, dtype=np.int64)[None, :]
    kvi = np.arange(128, dtype=np.int64)[:, None]
    stair = ((u - kvi) >= 384).astype(np.float32)          # [128, 896]

    q_w = np.asarray(q_w, dtype=np.float32)
    k_w = np.asarray(k_w, dtype=np.float32)
    v_w = np.asarray(v_w, dtype=np.float32)
    o_w = np.asarray(o_w, dtype=np.float32)

    in_maps = []
    for c in range(NCORES):
        wqkv = np.ascontiguousarray(np.concatenate(
            [q_w[:, c * DQ:(c + 1) * DQ],
             k_w[:, c * HD:(c + 1) * HD],
             v_w[:, c * HD:(c + 1) * HD]], axis=1))
        owc = np.ascontiguousarray(o_w[:, c * DQ:(c + 1) * DQ])
        in_maps.append({"xT": xT, "wqkv": wqkv, "ow": owc,
                        "cos_t": cos_t, "sin_t": sin_t, "stair": stair})
    return in_maps


def run(inputs: dict, trace: bool = False):
    """Run on the 8 NeuronCores; returns (full_output, BassKernelResults)."""
    nc = build_program()
    in_maps = make_in_maps(**inputs)
    res = run_bass_kernel_spmd(nc, in_maps, core_ids=list(range(NCORES)),
                               trace=trace)
    full = np.concatenate([res.results[c]["out"] for c in range(NCORES)], axis=1)
    return full.reshape(1, S, HIDDEN), res


def kernel(**inputs) -> np.ndarray:
    out, _ = run(inputs)
    return out


# revision 17
# speedup vs baseline: 1.2110x; 1.1444x over previous
# Mistral sliding-window attention (B=1, S=2048, H=4096, 32 q heads / 8 kv
# heads, window 4096 -> plain causal at this S) on 8 Trainium2 NeuronCores.
#
# Sharding: tensor-parallel over heads. Core c owns q heads 4c..4c+3 and kv
# head c. hidden_states is replicated (transposed on host to [H, S] so the
# contraction dim is the partition dim). Each core computes its attention
# output slice attn.T [512, S]; per-head AllGathers assemble the full
# [4096, S] while later heads still compute, and each core accumulates a
# 512-column slice of o_proj head-by-head; the host concatenates the 8
# column slices into the full output.
#
# All big matmuls run as float32r (fp32 storage, full-rate PE) with the
# moving dim = 512. Scores are computed transposed (S.T[kv, q]) so that the
# P@V contraction needs no transposes of the probability tiles; softmax
# denominators come from an all-ones stationary matmul accumulated alongside
# P@V, and the causal mask is a host-precomputed staircase slice multiplied
# in after exp. Attention runs two-pass per (head, q-chunk) — all score
# matmuls + exps first, then the PV/sum matmuls — so the PE never stalls on
# the ACT engine mid-chain.

from contextlib import ExitStack

import numpy as np

import concourse.bacc as bacc
import concourse.bass as bass
import concourse.mybir as mybir
import concourse.tile as tile
from concourse.bass_utils import run_bass_kernel_spmd
from concourse.masks import make_identity

HIDDEN = 4096
NH = 32
NKV = 8
HD = 128
THETA = 10000.0
S = 2048
NCORES = 8

QH = NH // NCORES          # 4 q heads per core
DQ = QH * HD               # 512 (per-core q/attn width)
DOUT = DQ + 2 * HD         # 768 = q heads + k + v projection width
MT = DOUT // 128           # 6 projection m-tiles (0..3 q, 4 k, 5 v)
KT = HIDDEN // 128         # 32 contraction tiles
KG = 4                     # x-load group: k-tiles per DMA
TCH = 512                  # token chunk (matmul moving dim)
NTCH = S // TCH            # 4
KVT = S // 128             # 16 kv tiles
SCALE = 1.0 / float(np.sqrt(HD))

F32 = mybir.dt.float32
F32R = mybir.dt.float32r
EXP = mybir.ActivationFunctionType.Exp


def _rope(nc, pool, src, dst, cs, sn):
    """dst = src*cos + rotate_half(src)*sin, in [d, tok] layout.

    src/dst are [128, n]; cs/sn are [64, n] (the two 64-row halves share
    frequencies). rotate_half: rows 0:64 get -src[64:128], rows 64:128 get
    src[0:64].
    """
    top, bot = src[0:64, :], src[64:128, :]
    ta = pool.tile([64, TCH], F32, name="rope_a")
    tb = pool.tile([64, TCH], F32, name="rope_b")
    nc.vector.tensor_mul(ta, top, cs)
    nc.vector.tensor_mul(tb, bot, sn)
    nc.vector.tensor_sub(dst[0:64, :], ta, tb)
    nc.vector.tensor_mul(ta, bot, cs)
    nc.vector.tensor_mul(tb, top, sn)
    nc.vector.tensor_add(dst[64:128, :], ta, tb)


def build_kernel_body(ctx: ExitStack, tc: tile.TileContext, outs, ins):
    nc = tc.nc
    xT, wqkv, ow, cos_t, sin_t, stair = (
        ins["xT"], ins["wqkv"], ins["ow"], ins["cos_t"], ins["sin_t"], ins["stair"],
    )
    out = outs["out"]

    # per-head bounce + gather buffers so each head's AllGather can fire as
    # soon as that head's attention is done (overlaps comm with compute)
    attn_loc = [nc.dram_tensor(f"attn_loc{h}", [HD, S], F32).ap()
                for h in range(QH)]
    attn_gat = [nc.dram_tensor(f"attn_gat{h}", [NCORES * HD, S], F32,
                               addr_space="Shared").ap()
                for h in range(QH)]

    singles = ctx.enter_context(tc.tile_pool(name="singles", bufs=1))
    cos_sb = singles.tile([64, S], F32)
    sin_sb = singles.tile([64, S], F32)
    stair_sb = singles.tile([128, 896], F32)
    nc.sync.dma_start(out=cos_sb, in_=cos_t)
    nc.sync.dma_start(out=sin_sb, in_=sin_t)
    nc.sync.dma_start(out=stair_sb, in_=stair)
    ones_sb = singles.tile([128, 128], F32R)
    ones_f = singles.tile([128, 128], F32)
    nc.vector.memset(ones_f, 1.0)
    nc.vector.tensor_copy(ones_sb, ones_f)
    ident_sb = singles.tile([128, 128], F32)
    make_identity(nc, ident_sb)

    # persistent projection outputs, [d, tok] layout
    qT = singles.tile([128, QH, S], F32R)    # q head h -> qT[:, h, :]
    kT = singles.tile([128, S], F32R)
    vT = singles.tile([128, S], F32)
    V = singles.tile([128, KVT, HD], F32R)   # V[:, j, :] = [tok 128, d 128]

    # ---- phase 1: QKV projection + RoPE --------------------------------
    with (
        tc.tile_pool(name="wq", bufs=1) as wp,
        tc.tile_pool(name="xt", bufs=3) as xp,
        tc.tile_pool(name="rope", bufs=2) as rp,
        tc.tile_pool(name="p1ps", bufs=1, space="PSUM") as pp1,
    ):
        # one DMA per k-tile (gpsimd queue) so the first matmul starts after
        # ~1/32 of the weight load instead of after the full 12 MB
        wq3 = wqkv.rearrange("(k p) d -> p k d", p=128)
        w_sb = []
        for k in range(KT):
            wk = wp.tile([128, DOUT], F32R, name=f"w{k}", tag=f"w{k}")
            nc.sync.dma_start(out=wk, in_=wq3[:, k, :])
            w_sb.append(wk)
        x3 = xT.rearrange("(k p) s -> p k s", p=128)
        for t in range(NTCH):
            ps = [pp1.tile([128, TCH], F32, name=f"p1_{m}", tag=f"p1_{m}")
                  for m in range(MT)]
            for kg in range(KT // KG):
                xg = xp.tile([128, KG, TCH], F32R, name="xg")
                nc.sync.dma_start(
                    out=xg,
                    in_=x3[:, kg * KG:(kg + 1) * KG, t * TCH:(t + 1) * TCH])
                for ki in range(KG):
                    k = kg * KG + ki
                    for m in range(MT):
                        nc.tensor.matmul(
                            ps[m],
                            lhsT=w_sb[k][:, m * 128:(m + 1) * 128],
                            rhs=xg[:, ki, :],
                            start=(k == 0), stop=(k == KT - 1),
                        )
            cs = cos_sb[:, t * TCH:(t + 1) * TCH]
            sn = sin_sb[:, t * TCH:(t + 1) * TCH]
            for h in range(QH):
                _rope(nc, rp, ps[h], qT[:, h, t * TCH:(t + 1) * TCH], cs, sn)
            _rope(nc, rp, ps[QH], kT[:, t * TCH:(t + 1) * TCH], cs, sn)
            nc.scalar.copy(out=vT[:, t * TCH:(t + 1) * TCH], in_=ps[QH + 1])
            # V = vT.T for this chunk's kv tiles (PE transpose [d,tok]->[tok,d])
            for j in range(4 * t, 4 * t + 4):
                pv = pp1.tile([128, 128], F32, name="pvt", tag="pvt")
                nc.tensor.transpose(pv, vT[:, j * 128:(j + 1) * 128], ident_sb)
                nc.scalar.copy(out=V[:, j, :], in_=pv)

    # ---- phases 2+3: attention heads with per-head AllGather, o_proj
    # head-partials interleaved one head behind -------------------------
    with (
        tc.tile_pool(name="pt", bufs=12) as ptp,
        tc.tile_pool(name="ao", bufs=2) as aop,
        tc.tile_pool(name="ow", bufs=16) as owp,
        tc.tile_pool(name="at", bufs=3) as atp,
        tc.tile_pool(name="acc", bufs=1) as accp,
        tc.tile_pool(name="p2sc", bufs=2, space="PSUM") as pp2,
        tc.tile_pool(name="p2acc", bufs=1, space="PSUM") as pa2,
        tc.tile_pool(name="p3ps", bufs=1, space="PSUM") as pp3,
    ):
        # o_proj output accumulator: acc[:, b, :] = out rows b*128:(b+1)*128
        acc = accp.tile([128, S // 128, TCH], F32)
        ow3 = ow.rearrange("(k p) d -> p k d", p=128)

        JW = 8  # kv tiles per pass-A/B wave (bounds live pt tiles)

        def attention_head(h):
            for c in range(NTCH):
                jmax = 4 * c + 3
                po = pa2.tile([128, TCH], F32, name="po", tag="po")
                psum_s = pa2.tile([128, TCH], F32, name="ps", tag="ps")
                qslice = qT[:, h, c * TCH:(c + 1) * TCH]
                for j0 in range(0, jmax + 1, JW):
                    j1 = min(j0 + JW, jmax + 1)
                    # pass A: scores + exp for this wave of kv tiles
                    pts = []
                    for j in range(j0, j1):
                        sc = pp2.tile([128, TCH], F32, name="sc", tag="sc")
                        nc.tensor.matmul(
                            sc, lhsT=kT[:, j * 128:(j + 1) * 128], rhs=qslice,
                            start=True, stop=True)
                        pt = ptp.tile([128, TCH], F32R, name="pt", tag="pt")
                        nc.scalar.activation(pt, sc, EXP, scale=SCALE)
                        rdiag = j - 4 * c
                        if rdiag >= 0:  # tile touches the causal diagonal
                            off = 384 - rdiag * 128
                            nc.vector.tensor_mul(
                                pt, pt, stair_sb[:, off:off + TCH])
                        pts.append(pt)
                    # pass B: PV + denominator accumulation
                    for i, j in enumerate(range(j0, j1)):
                        nc.tensor.matmul(po, lhsT=V[:, j, :], rhs=pts[i],
                                         start=(j == 0), stop=(j == jmax))
                        nc.tensor.matmul(psum_s, lhsT=ones_sb, rhs=pts[i],
                                         start=(j == 0), stop=(j == jmax))
                rec = aop.tile([128, TCH], F32, name="rec")
                nc.vector.reciprocal(rec, psum_s)
                ao = aop.tile([128, TCH], F32, name="ao")
                nc.vector.tensor_mul(ao, po, rec)
                nc.sync.dma_start(
                    out=attn_loc[h][:, c * TCH:(c + 1) * TCH], in_=ao)
            nc.gpsimd.collective_compute(
                "AllGather",
                mybir.AluOpType.bypass,
                ins=[attn_loc[h][:, :]],
                outs=[attn_gat[h][:, :]],
                replica_groups=[list(range(NCORES))],
            )

        def oproj_head(h):
            # o_proj partial for gathered head h: acc += sum_r at(r,h) @ ow(r,h)
            ows = []
            for r in range(NCORES):
                owk = owp.tile([128, DQ], F32R, name="owk", tag="owk")
                nc.sync.dma_start(out=owk, in_=ow3[:, r * QH + h, :])
                ows.append(owk)
            for g in range(S // TCH):
                pcs = [pp3.tile([128, TCH], F32, name=f"pc{i}", tag=f"pc{i}")
                       for i in range(4)]
                for r in range(NCORES):
                    at = atp.tile([128, TCH], F32R, name="at", tag="at")
                    nc.sync.dma_start(
                        out=at,
                        in_=attn_gat[h][r * 128:(r + 1) * 128,
                                        g * TCH:(g + 1) * TCH].bitcast(F32R))
                    for mi in range(4):
                        nc.tensor.matmul(
                            pcs[mi],
                            lhsT=at[:, mi * 128:(mi + 1) * 128],
                            rhs=ows[r],
                            start=(r == 0), stop=(r == NCORES - 1),
                        )
                for mi in range(4):
                    b = g * 4 + mi
                    if h == 0:
                        nc.scalar.copy(out=acc[:, b, :], in_=pcs[mi])
                    else:
                        nc.vector.tensor_add(acc[:, b, :], acc[:, b, :],
                                             pcs[mi])

        # interleave: o_proj for head h emitted after attention head h+1 so
        # its AllGather has landed by the time the PE reaches it
        attention_head(0)
        attention_head(1)
        oproj_head(0)
        attention_head(2)
        oproj_head(1)
        attention_head(3)
        oproj_head(2)
        oproj_head(3)

        nc.sync.dma_start(out=out.rearrange("(b p) d -> p b d", p=128), in_=acc)


_NC_CACHE = None


def build_program():
    global _NC_CACHE
    if _NC_CACHE is not None:
        return _NC_CACHE
    nc = bacc.Bacc("TRN2", target_bir_lowering=False, debug=False,
                   num_devices=NCORES)
    ins = {
        "xT": nc.dram_tensor("xT", [HIDDEN, S], F32R, kind="ExternalInput").ap(),
        "wqkv": nc.dram_tensor("wqkv", [HIDDEN, DOUT], F32R,
                               kind="ExternalInput").ap(),
        "ow": nc.dram_tensor("ow", [HIDDEN, DQ], F32R, kind="ExternalInput").ap(),
        "cos_t": nc.dram_tensor("cos_t", [64, S], F32, kind="ExternalInput").ap(),
        "sin_t": nc.dram_tensor("sin_t", [64, S], F32, kind="ExternalInput").ap(),
        "stair": nc.dram_tensor("stair", [128, 896], F32,
                                kind="ExternalInput").ap(),
    }
    outs = {"out": nc.dram_tensor("out", [S, DQ], F32, kind="ExternalOutput").ap()}
    with tile.TileContext(nc) as tc:
        with ExitStack() as ctx:
            build_kernel_body(ctx, tc, outs, ins)
    nc.compile()
    _NC_CACHE = nc
    return nc


def make_in_maps(hidden_states, position_ids, q_w, k_w, v_w, o_w):
    x = np.asarray(hidden_states, dtype=np.float32).reshape(S, HIDDEN)
    xT = np.ascontiguousarray(x.T)
    pos = np.asarray(position_ids).reshape(S).astype(np.float64)
    inv = 1.0 / (THETA ** (np.arange(0, HD, 2, dtype=np.float64) / HD))
    fr = inv[:, None] * pos[None, :]                       # [64, S]
    cos_t = np.cos(fr).astype(np.float32)
    sin_t = np.sin(fr).astype(np.float32)
    u = np.arange(896, dtype=np.int64)[None, :]
    kvi = np.arange(128, dtype=np.int64)[:, None]
    stair = ((u - kvi) >= 384).astype(np.float32)          # [128, 896]

    q_w = np.asarray(q_w, dtype=np.float32)
    k_w = np.asarray(k_w, dtype=np.float32)
    v_w = np.asarray(v_w, dtype=np.float32)
    o_w = np.asarray(o_w, dtype=np.float32)

    in_maps = []
    for c in range(NCORES):
        wqkv = np.ascontiguousarray(np.concatenate(
            [q_w[:, c * DQ:(c + 1) * DQ],
             k_w[:, c * HD:(c + 1) * HD],
             v_w[:, c * HD:(c + 1) * HD]], axis=1))
        owc = np.ascontiguousarray(o_w[:, c * DQ:(c + 1) * DQ])
        in_maps.append({"xT": xT, "wqkv": wqkv, "ow": owc,
                        "cos_t": cos_t, "sin_t": sin_t, "stair": stair})
    return in_maps


def run(inputs: dict, trace: bool = False):
    """Run on the 8 NeuronCores; returns (full_output, BassKernelResults)."""
    nc = build_program()
    in_maps = make_in_maps(**inputs)
    res = run_bass_kernel_spmd(nc, in_maps, core_ids=list(range(NCORES)),
                               trace=trace)
    full = np.concatenate([res.results[c]["out"] for c in range(NCORES)], axis=1)
    return full.reshape(1, S, HIDDEN), res


def kernel(**inputs) -> np.ndarray:
    out, _ = run(inputs)
    return out
